# revision 9
# baseline (speedup 1.0000x reference)
"""Weighted CE loss (histogram-binned class weights) on 8 Trainium2 cores.

Math rework of the reference:
  m_v   = argmax_c target[v, c]                 (per-voxel label)
  lse_v = log(sum_c exp(predict[v, c]))
  ce_v  = lse_v - predict[v, m_v]
  loss  = (1/V) * sum_v ce_v * w[m_v],  w_c = log(V / count_c) (count>0 else 1)

Regrouped by class:  loss = (1/V) * sum_c w_c * S_c   with
  S_c     = sum_{v: m_v = c} (lse_v - predict[v, c])
  count_c = #{v: m_v = c}

Sharding: batch dim B=8 over 8 cores (1 image each). Each core streams its
[C, H, W] slice once from HBM and emits per-class partials (count_c, S_c) as
[128, NCH, 2, C] column accumulators; the tiny [19]-vector combine (global
histogram "all-reduce" + log-weights + weighted sum) happens on host.
"""

import numpy as np

B, C, H, W = 8, 19, 512, 512
NCORES = 8
NCH = 8            # chunks per core
HB = H // NCH      # 64 h-rows per chunk
SPLIT = 2          # w-split folded into partitions
WP = W // SPLIT    # 256 free elements per partition
P = HB * SPLIT     # 128 partitions

_CACHE = {}


def _build_nc():
    import concourse.bass as bass
    import concourse.tile as tile
    from concourse import bacc, mybir
    from contextlib import ExitStack

    f32 = mybir.dt.float32
    Alu = mybir.AluOpType
    Act = mybir.ActivationFunctionType
    Ax = mybir.AxisListType

    nc = bacc.Bacc(
        "TRN2", target_bir_lowering=False, debug=False, num_devices=NCORES
    )
    pred = nc.dram_tensor("predict", [C, H, W], f32, kind="ExternalInput").ap()
    targ = nc.dram_tensor("target", [C, H, W], f32, kind="ExternalInput").ap()
    stats = nc.dram_tensor("stats", [NCH, P, 2, C], f32, kind="ExternalOutput").ap()

    pred_v = pred.rearrange("c (k h) (s w) -> k (h s) c w", k=NCH, s=SPLIT)
    targ_v = targ.rearrange("c (k h) (s w) -> k (h s) c w", k=NCH, s=SPLIT)

    with tile.TileContext(nc) as tc, ExitStack() as ctx:
        tin = ctx.enter_context(tc.tile_pool(name="tin", bufs=2))
        pin = ctx.enter_context(tc.tile_pool(name="pin", bufs=2))
        scratch = ctx.enter_context(tc.tile_pool(name="scratch", bufs=4))
        tree = ctx.enter_context(tc.tile_pool(name="tree", bufs=2))
        small = ctx.enter_context(tc.tile_pool(name="small", bufs=2))
        statp = ctx.enter_context(tc.tile_pool(name="statp", bufs=2))

        for k in range(NCH):
            T3 = tin.tile([P, C, WP], f32)
            nc.sync.dma_start(out=T3[:], in_=targ_v[k])
            P3 = pin.tile([P, C, WP], f32)
            nc.sync.dma_start(out=P3[:], in_=pred_v[k])

            # tmax = max over c of T3 (tree)
            t8 = tree.tile([P, 8, WP], f32, tag="t8")
            nc.vector.tensor_max(t8[:], T3[:, 0:8, :], T3[:, 8:16, :])
            t4 = tree.tile([P, 4, WP], f32, tag="t4")
            nc.vector.tensor_max(t4[:], t8[:, 0:4, :], t8[:, 4:8, :])
            t2 = tree.tile([P, 2, WP], f32, tag="t2")
            nc.vector.tensor_max(t2[:], t4[:, 0:2, :], t4[:, 2:4, :])
            ta = small.tile([P, 1, WP], f32, tag="ta")
            nc.vector.tensor_max(ta[:], t2[:, 0:1, :], t2[:, 1:2, :])
            tb = small.tile([P, 1, WP], f32, tag="tb")
            nc.vector.tensor_max(tb[:], ta[:], T3[:, 16:17, :])
            tc_ = small.tile([P, 1, WP], f32, tag="tc")
            nc.vector.tensor_max(tc_[:], tb[:], T3[:, 17:18, :])
            tmax = small.tile([P, 1, WP], f32, tag="tmax")
            nc.vector.tensor_max(tmax[:], tc_[:], T3[:, 18:19, :])

            # E3 = exp(P3); sexp = sum over c (tree); lse = ln(sexp)
            E3 = scratch.tile([P, C, WP], f32, tag="big")
            nc.scalar.activation(E3[:], P3[:], Act.Exp)
            e8 = tree.tile([P, 8, WP], f32, tag="t8")
            nc.vector.tensor_add(e8[:], E3[:, 0:8, :], E3[:, 8:16, :])
            e4 = tree.tile([P, 4, WP], f32, tag="t4")
            nc.vector.tensor_add(e4[:], e8[:, 0:4, :], e8[:, 4:8, :])
            e2 = tree.tile([P, 2, WP], f32, tag="t2")
            nc.vector.tensor_add(e2[:], e4[:, 0:2, :], e4[:, 2:4, :])
            ea = small.tile([P, 1, WP], f32, tag="ta")
            nc.vector.tensor_add(ea[:], e2[:, 0:1, :], e2[:, 1:2, :])
            eb = small.tile([P, 1, WP], f32, tag="tb")
            nc.vector.tensor_add(eb[:], ea[:], E3[:, 16:17, :])
            ec = small.tile([P, 1, WP], f32, tag="tc")
            nc.vector.tensor_add(ec[:], eb[:], E3[:, 17:18, :])
            sexp = small.tile([P, 1, WP], f32, tag="sexp")
            nc.vector.tensor_add(sexp[:], ec[:], E3[:, 18:19, :])
            lse = small.tile([P, 1, WP], f32, tag="lse")
            nc.scalar.activation(lse[:], sexp[:], Act.Ln)

            # ind3 = (T3 >= tmax)  [one-hot by class, ties ~measure-zero]
            ind3 = scratch.tile([P, C, WP], f32, tag="big")
            nc.vector.tensor_tensor(
                ind3[:], T3[:], tmax.broadcast_to([P, C, WP]), Alu.is_ge
            )
            # D3 = lse - P3
            D3 = scratch.tile([P, C, WP], f32, tag="big")
            nc.vector.tensor_tensor(
                D3[:], lse.broadcast_to([P, C, WP]), P3[:], Alu.subtract
            )
            # PR3 = D3 * ind3
            PR3 = scratch.tile([P, C, WP], f32, tag="big")
            nc.vector.tensor_mul(PR3[:], D3[:], ind3[:])

            # per-class partial sums over w
            stat_t = statp.tile([P, 2, C], f32)
            nc.vector.tensor_reduce(stat_t[:, 0, :], ind3[:], axis=Ax.X, op=Alu.add)
            nc.vector.tensor_reduce(stat_t[:, 1, :], PR3[:], axis=Ax.X, op=Alu.add)
            nc.sync.dma_start(out=stats[k], in_=stat_t[:])

    nc.compile()
    return nc


def _get_nc():
    if "nc" not in _CACHE:
        _CACHE["nc"] = _build_nc()
    return _CACHE["nc"]


def kernel(predict: np.ndarray, target: np.ndarray) -> np.ndarray:
    from concourse.bass_utils import run_bass_kernel_spmd

    nc = _get_nc()
    predict = np.asarray(predict, dtype=np.float32)
    target = np.asarray(target, dtype=np.float32)
    in_maps = [
        {
            "predict": np.ascontiguousarray(predict[i]),
            "target": np.ascontiguousarray(target[i]),
        }
        for i in range(NCORES)
    ]
    res = run_bass_kernel_spmd(nc, in_maps, list(range(NCORES))).results

    cnt = np.zeros(C, np.float64)
    S = np.zeros(C, np.float64)
    for r in res:
        st = np.asarray(r["stats"], dtype=np.float64)  # [NCH, P, 2, C]
        cnt += st[:, :, 0, :].sum(axis=(0, 1))
        S += st[:, :, 1, :].sum(axis=(0, 1))

    V = float(B * H * W)
    w = np.where(cnt > 0.0, np.log(V / np.maximum(cnt, 1.0)), 1.0)
    ans = float((S * w).sum() / V)
    return np.float32(ans)


# revision 11
# speedup vs baseline: 74.1297x; 74.1297x over previous
"""Weighted CE loss (histogram-binned class weights) on 8 Trainium2 cores.

Math rework of the reference:
  m_v   = argmax_c target[v, c]                 (per-voxel label)
  lse_v = log(sum_c exp(predict[v, c]))
  ce_v  = lse_v - predict[v, m_v]
  loss  = (1/V) * sum_v ce_v * w[m_v],  w_c = log(V / count_c) (count>0 else 1)

Regrouped by class:  loss = (1/V) * sum_c w_c * S_c   with
  S_c     = sum_{v: m_v = c} (lse_v - predict[v, c])
  count_c = #{v: m_v = c}

Sharding: batch dim B=8 over 8 cores (1 image each). Each core streams its
[C, H, W] slice once from HBM and emits per-class partials (count_c, S_c) as
[128, NCH, 2, C] column accumulators; the tiny [19]-vector combine (global
histogram "all-reduce" + log-weights + weighted sum) happens on host.
"""

import numpy as np

B, C, H, W = 8, 19, 512, 512
NCORES = 8
NCH = 8            # chunks per core
HB = H // NCH      # 64 h-rows per chunk
SPLIT = 2          # w-split folded into partitions
WP = W // SPLIT    # 256 free elements per partition
P = HB * SPLIT     # 128 partitions

_CACHE = {}


def _build_nc(repeat: int = 1):
    import concourse.bass as bass
    import concourse.tile as tile
    from concourse import bacc, mybir
    from contextlib import ExitStack

    f32 = mybir.dt.float32
    Alu = mybir.AluOpType
    Act = mybir.ActivationFunctionType
    Ax = mybir.AxisListType

    nc = bacc.Bacc(
        "TRN2", target_bir_lowering=False, debug=False, num_devices=NCORES
    )
    pred = nc.dram_tensor("predict", [C, H, W], f32, kind="ExternalInput").ap()
    targ = nc.dram_tensor("target", [C, H, W], f32, kind="ExternalInput").ap()
    stats = nc.dram_tensor("stats", [NCH, P, 2, C], f32, kind="ExternalOutput").ap()

    pred_v = pred.rearrange("c (k h) (s w) -> k (h s) c w", k=NCH, s=SPLIT)
    targ_v = targ.rearrange("c (k h) (s w) -> k (h s) c w", k=NCH, s=SPLIT)

    with tile.TileContext(nc) as tc, ExitStack() as ctx:
        tin = ctx.enter_context(tc.tile_pool(name="tin", bufs=2))
        pin = ctx.enter_context(tc.tile_pool(name="pin", bufs=2))
        scratch = ctx.enter_context(tc.tile_pool(name="scratch", bufs=4))
        tree = ctx.enter_context(tc.tile_pool(name="tree", bufs=2))
        small = ctx.enter_context(tc.tile_pool(name="small", bufs=2))
        statp = ctx.enter_context(tc.tile_pool(name="statp", bufs=2))

        for kk in range(NCH * repeat):
            k = kk % NCH
            T3 = tin.tile([P, C, WP], f32)
            nc.sync.dma_start(out=T3[:], in_=targ_v[k])
            P3 = pin.tile([P, C, WP], f32)
            nc.sync.dma_start(out=P3[:], in_=pred_v[k])

            # tmax = max over c of T3 (tree)
            t8 = tree.tile([P, 8, WP], f32, tag="t8")
            nc.vector.tensor_max(t8[:], T3[:, 0:8, :], T3[:, 8:16, :])
            t4 = tree.tile([P, 4, WP], f32, tag="t4")
            nc.vector.tensor_max(t4[:], t8[:, 0:4, :], t8[:, 4:8, :])
            t2 = tree.tile([P, 2, WP], f32, tag="t2")
            nc.vector.tensor_max(t2[:], t4[:, 0:2, :], t4[:, 2:4, :])
            ta = small.tile([P, 1, WP], f32, tag="ta")
            nc.vector.tensor_max(ta[:], t2[:, 0:1, :], t2[:, 1:2, :])
            tb = small.tile([P, 1, WP], f32, tag="tb")
            nc.vector.tensor_max(tb[:], ta[:], T3[:, 16:17, :])
            tc_ = small.tile([P, 1, WP], f32, tag="tc")
            nc.vector.tensor_max(tc_[:], tb[:], T3[:, 17:18, :])
            tmax = small.tile([P, 1, WP], f32, tag="tmax")
            nc.vector.tensor_max(tmax[:], tc_[:], T3[:, 18:19, :])

            # E3 = exp(P3); sexp = sum over c (tree); lse = ln(sexp)
            E3 = scratch.tile([P, C, WP], f32, tag="big")
            nc.scalar.activation(E3[:], P3[:], Act.Exp)
            e8 = tree.tile([P, 8, WP], f32, tag="t8")
            nc.vector.tensor_add(e8[:], E3[:, 0:8, :], E3[:, 8:16, :])
            e4 = tree.tile([P, 4, WP], f32, tag="t4")
            nc.vector.tensor_add(e4[:], e8[:, 0:4, :], e8[:, 4:8, :])
            e2 = tree.tile([P, 2, WP], f32, tag="t2")
            nc.vector.tensor_add(e2[:], e4[:, 0:2, :], e4[:, 2:4, :])
            ea = small.tile([P, 1, WP], f32, tag="ta")
            nc.vector.tensor_add(ea[:], e2[:, 0:1, :], e2[:, 1:2, :])
            eb = small.tile([P, 1, WP], f32, tag="tb")
            nc.vector.tensor_add(eb[:], ea[:], E3[:, 16:17, :])
            ec = small.tile([P, 1, WP], f32, tag="tc")
            nc.vector.tensor_add(ec[:], eb[:], E3[:, 17:18, :])
            sexp = small.tile([P, 1, WP], f32, tag="sexp")
            nc.vector.tensor_add(sexp[:], ec[:], E3[:, 18:19, :])
            lse = small.tile([P, 1, WP], f32, tag="lse")
            nc.scalar.activation(lse[:], sexp[:], Act.Ln)

            # ind3 = (T3 >= tmax)  [one-hot by class, ties ~measure-zero]
            ind3 = scratch.tile([P, C, WP], f32, tag="big")
            nc.vector.tensor_tensor(
                ind3[:], T3[:], tmax.broadcast_to([P, C, WP]), Alu.is_ge
            )
            # D3 = lse - P3
            D3 = scratch.tile([P, C, WP], f32, tag="big")
            nc.vector.tensor_tensor(
                D3[:], lse.broadcast_to([P, C, WP]), P3[:], Alu.subtract
            )
            # PR3 = D3 * ind3
            PR3 = scratch.tile([P, C, WP], f32, tag="big")
            nc.vector.tensor_mul(PR3[:], D3[:], ind3[:])

            # per-class partial sums over w
            stat_t = statp.tile([P, 2, C], f32)
            nc.vector.tensor_reduce(stat_t[:, 0, :], ind3[:], axis=Ax.X, op=Alu.add)
            nc.vector.tensor_reduce(stat_t[:, 1, :], PR3[:], axis=Ax.X, op=Alu.add)
            nc.sync.dma_start(out=stats[k], in_=stat_t[:])

    nc.compile()
    return nc


def _get_nc():
    if "nc" not in _CACHE:
        _CACHE["nc"] = _build_nc()
    return _CACHE["nc"]


def kernel(predict: np.ndarray, target: np.ndarray) -> np.ndarray:
    from concourse.bass_utils import run_bass_kernel_spmd

    nc = _get_nc()
    predict = np.asarray(predict, dtype=np.float32)
    target = np.asarray(target, dtype=np.float32)
    in_maps = [
        {
            "predict": np.ascontiguousarray(predict[i]),
            "target": np.ascontiguousarray(target[i]),
        }
        for i in range(NCORES)
    ]
    res = run_bass_kernel_spmd(nc, in_maps, list(range(NCORES))).results

    cnt = np.zeros(C, np.float64)
    S = np.zeros(C, np.float64)
    for r in res:
        st = np.asarray(r["stats"], dtype=np.float64)  # [NCH, P, 2, C]
        cnt += st[:, :, 0, :].sum(axis=(0, 1))
        S += st[:, :, 1, :].sum(axis=(0, 1))

    V = float(B * H * W)
    w = np.where(cnt > 0.0, np.log(V / np.maximum(cnt, 1.0)), 1.0)
    ans = float((S * w).sum() / V)
    return np.float32(ans)
